# revision 7
# baseline (speedup 1.0000x reference)
"""Trainium2 Bass kernel for the quirky-reshape 16-head attention layer.

Shapes (hardcoded): x [2, 2048, 1024], Wq/Wk/Wv/Wo [1024, 1024], n_head=16.

Sharding: core c in [0,8) handles batch b=c//4 and head group g=c%4 (heads
4g..4g+3). The reference's quirky `qkv.reshape(b, s, d)` merge makes output
rows [h*128, (h+1)*128) depend only on head h, so each core produces the
disjoint output row block [g*512, (g+1)*512) of its batch — no collectives.

Precision: q/k path (projections + scores) in fp16, exp / AV / O-projection
in bf16 (fp32 range needed: exp values reach ~1e30), all matmul accumulation
in fp32 PSUM.

Per-core dataflow (transposed-scores streaming attention, ACT-rate paced):
  For each head pair (2 pairs of 2 heads), for each 512-query window (4),
  stream over 16 key blocks kb:
    S^T[kb]  = [kA^T qA | kB^T qB]   two K=64 row-tiled matmuls running
               concurrently in the upper/lower PE array halves (tile_position
               derived from partition ranges) -> one PSUM [128, 1024] fp32
    E[kb]    = exp(S^T[kb])          one ScalarE ACTIVATE over both heads
    AV[h]   += [ones|v_h]^T E[kb,h]  one kb behind the exp pipeline; rows
               0:64 accumulate the softmax denominator, 64:128 the numerator
  Window drain: rcp = reciprocal(denom); broadcast rcp to partitions 64:128
  via DMA; Qs[64:128, hg, q] = qkv * rcp (contiguous bf16); Qs[0:64, hg, q+1]
  = shift-by-one DMA copy of the upper half. The quirky merge then reduces to
  a stride-16 stationary read in the O-projection:
    out_hg = sum_kt Qs[:, hg, (2kt+1)::16]^T Wo[kt]
  QKV projections and O-projections are emitted as single-matmul "filler
  atoms" paced into the attention loop's PE slack (the loop is ACT-bound),
  with deadline-ordered scheduling; input DMAs are chunked and
  priority-ordered so the first projection starts ~4us in.
"""

import numpy as np

B, S, D, H = 2, 2048, 1024, 16
DH = 64
NCORES = 8

_CACHE = {}


def _build_program():
    from concourse import bacc, tile, mybir

    F32 = mybir.dt.float32
    F16 = mybir.dt.float16
    BF16 = mybir.dt.bfloat16
    EXP = mybir.ActivationFunctionType.Exp

    nc = bacc.Bacc(None, target_bir_lowering=False, debug=False)

    xt_d = nc.dram_tensor("xt", [128, 8, 2048], F16, kind="ExternalInput").ap()
    wq_d = nc.dram_tensor("wq", [128, 8, 256], F16, kind="ExternalInput").ap()
    wk_d = nc.dram_tensor("wk", [128, 8, 256], F16, kind="ExternalInput").ap()
    wv_d = nc.dram_tensor("wv", [128, 8, 256], F16, kind="ExternalInput").ap()
    wo_d = nc.dram_tensor("wo", [128, 8, 1024], BF16, kind="ExternalInput").ap()
    out_d = nc.dram_tensor("out", [4, 128, 1024], F32, kind="ExternalOutput").ap()

    with tile.TileContext(nc) as tc:
        with (
            tc.tile_pool(name="keep", bufs=1) as keep,
            tc.tile_pool(name="exp", bufs=6) as expp,
            tc.tile_pool(name="rcp", bufs=2) as rcpp,
            tc.tile_pool(name="osb", bufs=2) as osbp,
            tc.tile_pool(name="ps", bufs=1, space="PSUM") as psp,
        ):
            # ------- persistent SBUF tiles -------
            xt = [[keep.tile([128, 512], F16, tag=f"xt{kt}_{c}", name=f"xt{kt}_{c}")
                   for c in range(4)] for kt in range(8)]
            wqt = keep.tile([128, 8, 256], F16, tag="wq", name="wqt")
            wkt = keep.tile([128, 8, 256], F16, tag="wk", name="wkt")
            wvt = keep.tile([128, 8, 256], F16, tag="wv", name="wvt")
            wot = keep.tile([128, 8, 1024], BF16, tag="wo", name="wot")
            qT = [keep.tile([128, 2048], F16, tag=f"qT{p}", name=f"qT{p}")
                  for p in range(2)]
            kT = [keep.tile([128, 2048], F16, tag=f"kT{p}", name=f"kT{p}")
                  for p in range(2)]
            v_sb = [keep.tile([128, 4, 128], BF16, tag=f"v{kb}", name=f"v{kb}")
                    for kb in range(16)]
            qs = keep.tile([128, 4, 2048], BF16, tag="qs", name="qs")

            # ------- input DMAs: chunked, priority-ordered -------
            # first projection needs wq + xt chunk 0; weights on scalar (only
            # early use), xt chunks round-robin on sync/gpsimd/vector.
            nc.scalar.dma_start(out=wqt[:], in_=wq_d[:])
            nc.scalar.dma_start(out=wkt[:], in_=wk_d[:])
            nc.scalar.dma_start(out=wvt[:], in_=wv_d[:])
            for c in range(4):
                for kt in range(8):
                    eng = nc.sync if kt % 2 == 0 else nc.gpsimd
                    eng.dma_start(out=xt[kt][c][:],
                                  in_=xt_d[:, kt, c * 512:(c + 1) * 512])
            nc.scalar.dma_start(out=wot[:], in_=wo_d[:])

            # ones rows for the denominator trick (disjoint from the V copy)
            for kb in range(16):
                nc.gpsimd.memset(v_sb[kb][:, :, 0:64], 1.0)

            # ------- matmul-group emit helpers (atom-granular) -------
            def qk_atoms(nm, pair, ch):
                """Project q or k for one 512-token chunk: 8 accumulating
                matmuls (one atom each) + a PSUM->SBUF cast."""
                wt = wqt if nm == "q" else wkt
                dst = qT[pair] if nm == "q" else kT[pair]
                st = {}

                def mk(kt):
                    def f():
                        if kt == 0:
                            st["ps"] = psp.tile([128, 512], F32, tag="fill",
                                                bufs=1, name="qkps")
                        nc.tensor.matmul(
                            st["ps"][:],
                            wt[:, kt, pair * 128:(pair + 1) * 128],
                            xt[kt][ch][:],
                            start=(kt == 0),
                            stop=(kt == 7),
                        )
                        if kt == 7:
                            nc.vector.tensor_copy(
                                dst[:, ch * 512:(ch + 1) * 512], st["ps"][:])
                    return f
                return [(216, mk(kt)) for kt in range(8)]

            def v_atoms(kb):
                """V projection for one 128-token key block: 2 atoms of 4
                matmuls (N=256) + cast into the [ones|v] tile."""
                st = {}

                def mk(half):
                    def f():
                        if half == 0:
                            st["ps"] = psp.tile([128, 512], F32, tag="fill",
                                                bufs=1, name="vps")
                        for kt in range(4 * half, 4 * half + 4):
                            nc.tensor.matmul(
                                st["ps"][:, 0:256],
                                xt[kt][kb // 4][:, (kb % 4) * 128:(kb % 4 + 1) * 128],
                                wvt[:, kt, :],
                                start=(kt == 0),
                                stop=(kt == 7),
                            )
                        if half == 1:
                            nc.vector.tensor_copy(
                                v_sb[kb][:, :, 64:128],
                                st["ps"][:, 0:256].rearrange(
                                    "p (a b) -> p a b", a=4))
                    return f
                return [(436, mk(0)), (436, mk(1))]

            def oproj_atoms(hg):
                """O-projection for head group hg: 2 column halves x 8
                accumulating matmuls with stride-16 stationary reads of qs."""
                atoms = []
                qs_h = qs[:, hg, :].rearrange("p (r t) -> p r t", t=16)
                for h in range(2):
                    st = {}

                    def mk(kt, h=h, st=st):
                        def f():
                            if kt == 0:
                                st["ps"] = psp.tile([128, 512], F32, tag="fill",
                                                    bufs=1, name="ops")
                            nc.tensor.matmul(
                                st["ps"][:],
                                qs_h[:, :, 2 * kt + 1],
                                wot[:, kt, h * 512:(h + 1) * 512],
                                start=(kt == 0),
                                stop=(kt == 7),
                            )
                            if kt == 7:
                                ot = osbp.tile([128, 512], F32, tag="ot", name="ot")
                                nc.vector.tensor_copy(ot[:], st["ps"][:])
                                nc.sync.dma_start(
                                    out=out_d[hg, :, h * 512:(h + 1) * 512],
                                    in_=ot[:])
                        return f
                    atoms += [(216, mk(kt)) for kt in range(8)]
                return atoms

            # budget-paced filler scheduler (used from window 1 on)
            from collections import deque
            fq = deque()
            bstate = {"b": 0.0}

            def sched_add(gid, atoms):
                for c, fn in atoms:
                    fq.append((c, fn, gid))

            def sched_step(budget=360.0):
                bstate["b"] += budget
                while fq and fq[0][0] <= bstate["b"]:
                    c, fn, _ = fq.popleft()
                    bstate["b"] -= c
                    fn()

            def sched_require(*gids):
                """Force-emit queued groups up to and including the given
                gids (compile-order backstop: the PE stream is in-order, so
                prerequisites must be emitted before their consumers)."""
                want = set(gids)
                while want & {g for _, _, g in fq}:
                    c, fn, g = fq.popleft()
                    fn()

            def sched_flush():
                while fq:
                    fq.popleft()[1]()
                bstate["b"] = 0.0

            # ------- attention window -------
            def attn_window(pair, qc, w0_fillers=None):
                q0 = qc * 512
                av = {hl: psp.tile([128, 512], F32, tag="av", bufs=3,
                                   name=f"av{hl}")
                      for hl in range(2)}
                prev_et = None

                def av_mms(kb, et):
                    for hl in range(2):
                        nc.tensor.matmul(
                            av[hl][:],
                            v_sb[kb][:, 2 * pair + hl, :],
                            et[:, hl * 512:(hl + 1) * 512],
                            start=(kb == 0),
                            stop=(kb == 15),
                        )

                for kb in range(16):
                    sc = psp.tile([128, 1024], F32, tag="sc", bufs=2, name="sc")
                    for hl in range(2):
                        h0 = hl * 64
                        nc.tensor.matmul(
                            sc[:, hl * 512:(hl + 1) * 512],
                            kT[pair][h0:h0 + 64, kb * 128:(kb + 1) * 128],
                            qT[pair][h0:h0 + 64, q0:q0 + 512],
                            start=True,
                            stop=True,
                        )
                    if prev_et is not None:
                        av_mms(kb - 1, prev_et)
                    et = expp.tile([128, 1024], BF16, tag="exp", name="et")
                    nc.scalar.activation(et[:], sc[:], EXP)
                    prev_et = et
                    if w0_fillers is not None:
                        for fn in w0_fillers.get(kb, []):
                            fn()
                    else:
                        sched_step()
                av_mms(15, prev_et)

                # window drain: normalize into qs + shifted lower copy
                for hl in range(2):
                    hg = 2 * pair + hl
                    ap = av[hl]
                    rt = rcpp.tile([128, 512], F32, tag="rcp", name="rt")
                    nc.vector.reciprocal_approx_fast(rt[0:64, :], ap[0:64, :])
                    nc.sync.dma_start(out=rt[64:128, :], in_=rt[0:64, :])
                    nc.vector.tensor_mul(
                        qs[64:128, hg, q0:q0 + 512],
                        ap[64:128, :],
                        rt[64:128, :],
                    )
                    n = 511 if qc == 3 else 512
                    nc.gpsimd.dma_start(
                        out=qs[0:64, hg, q0 + 1:q0 + 1 + n],
                        in_=qs[64:128, hg, q0:q0 + n],
                    )

            # ------- pre-phase: minimum to start pair-0 window 0 -------
            # (fill-ring groups must never interleave: each group's atoms
            # are emitted contiguously)
            for _, fn in qk_atoms("q", 0, 0):
                fn()
            for _, fn in qk_atoms("k", 0, 0):
                fn()

            # ------- window 0 of pair 0: deadline-driven fillers -------
            # v(kb) must land before iter kb+1; kT chunk c before iter 4c;
            # qT chunk 1 before window 1. Whole groups per iteration.
            w0 = {kb: [] for kb in range(16)}
            w0[0] = [v_atoms(0), v_atoms(1), v_atoms(2)]
            w0[1] = [v_atoms(3)]
            w0[2] = [v_atoms(4), v_atoms(5)]
            w0[3] = [qk_atoms("k", 0, 1)]
            w0[4] = [v_atoms(6), v_atoms(7)]
            w0[5] = [v_atoms(8)]
            w0[6] = [qk_atoms("k", 0, 2)]
            w0[7] = [v_atoms(9), v_atoms(10)]
            w0[8] = [v_atoms(11)]
            w0[9] = [v_atoms(12)]
            w0[10] = [qk_atoms("k", 0, 3)]
            w0[11] = [v_atoms(13)]
            w0[12] = [v_atoms(14)]
            w0[13] = [v_atoms(15)]
            w0[14] = [qk_atoms("q", 0, 1)]
            w0f = {kb: [fn for grp in lst for _, fn in grp]
                   for kb, lst in w0.items()}
            attn_window(0, 0, w0_fillers=w0f)

            # ------- windows 1..7: budget-paced fillers, deadline order ----
            sched_add("q02", qk_atoms("q", 0, 2))    # by iter 32
            sched_add("k10", qk_atoms("k", 1, 0))    # by iter 64
            sched_add("q03", qk_atoms("q", 0, 3))    # by iter 48
            sched_add("k11", qk_atoms("k", 1, 1))    # by iter 68
            sched_add("k12", qk_atoms("k", 1, 2))    # by iter 72
            sched_add("k13", qk_atoms("k", 1, 3))    # by iter 76
            sched_add("q10", qk_atoms("q", 1, 0))    # by iter 64
            sched_add("q11", qk_atoms("q", 1, 1))    # by iter 80
            sched_add("q12", qk_atoms("q", 1, 2))    # by iter 96
            sched_add("q13", qk_atoms("q", 1, 3))    # by iter 112

            attn_window(0, 1)
            sched_require("q02")
            attn_window(0, 2)
            sched_require("q03")
            attn_window(0, 3)
            # pair-0 qs complete after the drains above -> O-proj becomes
            # available as filler work during pair 1.
            sched_require("k10", "k11", "k12", "k13", "q10")
            sched_add("op0", oproj_atoms(0))
            sched_add("op1", oproj_atoms(1))
            attn_window(1, 0)
            sched_require("q11")
            attn_window(1, 1)
            sched_require("q12")
            attn_window(1, 2)
            sched_require("q13")
            attn_window(1, 3)

            # ------- tail -------
            sched_flush()
            for _, fn in oproj_atoms(2):
                fn()
            for _, fn in oproj_atoms(3):
                fn()

    nc.compile()
    return nc


def _get_program():
    if "nc" not in _CACHE:
        _CACHE["nc"] = _build_program()
    return _CACHE["nc"]


def _make_in_maps(x, Wq, Wk, Wv, Wo):
    import ml_dtypes

    bf16 = ml_dtypes.bfloat16
    wo8 = np.ascontiguousarray(
        Wo.astype(bf16).reshape(8, 128, 1024).transpose(1, 0, 2))
    xts = [
        np.ascontiguousarray(
            x[b].T.astype(np.float16).reshape(8, 128, 2048).transpose(1, 0, 2))
        for b in range(B)
    ]
    wq16 = Wq.astype(np.float16)
    wk16 = Wk.astype(np.float16)
    wv16 = Wv.astype(np.float16)
    def pack(w, cols):
        return np.ascontiguousarray(
            w[:, cols].reshape(8, 128, 256).transpose(1, 0, 2))
    in_maps = []
    for c in range(NCORES):
        b, g = c // 4, c % 4
        cols = slice(4 * g * DH, 4 * (g + 1) * DH)
        in_maps.append(
            {
                "xt": xts[b],
                "wq": pack(wq16, cols),
                "wk": pack(wk16, cols),
                "wv": pack(wv16, cols),
                "wo": wo8,
            }
        )
    return in_maps


def kernel(x, Wq, Wk, Wv, Wo, n_head):
    from concourse.bass_utils import run_bass_kernel_spmd

    assert int(n_head) == H
    x = np.asarray(x, np.float32)
    Wq = np.asarray(Wq, np.float32)
    Wk = np.asarray(Wk, np.float32)
    Wv = np.asarray(Wv, np.float32)
    Wo = np.asarray(Wo, np.float32)

    nc = _get_program()
    in_maps = _make_in_maps(x, Wq, Wk, Wv, Wo)
    res = run_bass_kernel_spmd(nc, in_maps, list(range(NCORES)))

    out = np.empty((B, S, D), np.float32)
    for c in range(NCORES):
        b, g = c // 4, c % 4
        out[b, g * 512:(g + 1) * 512, :] = res.results[c]["out"].reshape(512, 1024)
    return out


# revision 14
# speedup vs baseline: 1.1400x; 1.1400x over previous
"""Trainium2 Bass kernel for the quirky-reshape 16-head attention layer.

Shapes (hardcoded): x [2, 2048, 1024], Wq/Wk/Wv/Wo [1024, 1024], n_head=16.

Sharding: core c in [0,8) handles batch b=c//4 and head group g=c%4 (heads
4g..4g+3). The reference's quirky `qkv.reshape(b, s, d)` merge makes output
rows [h*128, (h+1)*128) depend only on head h, so each core produces the
disjoint output row block [g*512, (g+1)*512) of its batch — no collectives.

Precision: q/k path (projections + scores) in fp16, exp / AV / O-projection
in bf16 (fp32 range needed: exp values reach ~1e30), all matmul accumulation
in fp32 PSUM.

Per-core dataflow (transposed-scores streaming attention, ACT-rate paced):
  For each head pair (2 pairs of 2 heads), for each 512-query window (4),
  stream over 16 key blocks kb:
    S^T[kb]  = [kA^T qA | kB^T qB]   two K=64 row-tiled matmuls running
               concurrently in the upper/lower PE array halves (tile_position
               derived from partition ranges) -> one PSUM [128, 1024] fp32
    E[kb]    = exp(S^T[kb])          one ScalarE ACTIVATE over both heads
    AV[h]   += [ones|v_h]^T E[kb,h]  one kb behind the exp pipeline; rows
               0:64 accumulate the softmax denominator, 64:128 the numerator
  Window drain: rcp = reciprocal(denom); broadcast rcp to partitions 64:128
  via DMA; Qs[64:128, hg, q] = qkv * rcp (contiguous bf16); Qs[0:64, hg, q+1]
  = shift-by-one DMA copy of the upper half. The quirky merge then reduces to
  a stride-16 stationary read in the O-projection:
    out_hg = sum_kt Qs[:, hg, (2kt+1)::16]^T Wo[kt]
  QKV projections and O-projections are emitted as single-matmul "filler
  atoms" paced into the attention loop's PE slack (the loop is ACT-bound),
  with deadline-ordered scheduling; input DMAs are chunked and
  priority-ordered so the first projection starts ~4us in.
"""

import numpy as np

B, S, D, H = 2, 2048, 1024, 16
DH = 64
NCORES = 8

_CACHE = {}


def _build_program():
    from concourse import bacc, tile, mybir

    F32 = mybir.dt.float32
    F16 = mybir.dt.float16
    BF16 = mybir.dt.bfloat16
    EXP = mybir.ActivationFunctionType.Exp

    nc = bacc.Bacc(None, target_bir_lowering=False, debug=False)

    xt_d = nc.dram_tensor("xt", [128, 8, 2048], F16, kind="ExternalInput").ap()
    wq_d = nc.dram_tensor("wq", [128, 2, 8, 128], F16, kind="ExternalInput").ap()
    wk_d = nc.dram_tensor("wk", [128, 2, 8, 128], F16, kind="ExternalInput").ap()
    wv_d = nc.dram_tensor("wv", [128, 8, 256], F16, kind="ExternalInput").ap()
    wo_d = nc.dram_tensor("wo", [128, 8, 1024], BF16, kind="ExternalInput").ap()
    out_d = nc.dram_tensor("out", [4, 128, 1024], F32, kind="ExternalOutput").ap()

    with tile.TileContext(nc) as tc:
        with (
            tc.tile_pool(name="keep", bufs=1) as keep,
            tc.tile_pool(name="exp", bufs=6) as expp,
            tc.tile_pool(name="rcp", bufs=2) as rcpp,
            tc.tile_pool(name="osb", bufs=2) as osbp,
            tc.tile_pool(name="ps", bufs=1, space="PSUM") as psp,
        ):
            # ------- persistent SBUF tiles -------
            xt = [[keep.tile([128, 512], F16, tag=f"xt{kt}_{c}", name=f"xt{kt}_{c}")
                   for c in range(4)] for kt in range(8)]
            wqt = keep.tile([128, 2, 8, 128], F16, tag="wq", name="wqt")
            wkt = keep.tile([128, 2, 8, 128], F16, tag="wk", name="wkt")
            wvt = keep.tile([128, 8, 256], F16, tag="wv", name="wvt")
            wot = keep.tile([128, 8, 1024], BF16, tag="wo", name="wot")
            qT = [keep.tile([128, 2048], F16, tag=f"qT{p}", name=f"qT{p}")
                  for p in range(2)]
            kT = [keep.tile([128, 2048], F16, tag=f"kT{p}", name=f"kT{p}")
                  for p in range(2)]
            vbig = keep.tile([128, 16, 4, 128], BF16, tag="vbig", name="vbig")
            v_sb = [vbig[:, kb] for kb in range(16)]
            qs = keep.tile([128, 4, 2048], BF16, tag="qs", name="qs")

            # ------- input DMAs: chunked, priority-ordered -------
            # scores(0) needs wq + wk + xt chunk 0 (~2.5 MB): wq/wv on
            # scalar, xt c0 split across sync/gpsimd with wk halves right
            # behind, remaining xt chunks and wo streaming after.
            # scalar = fast software-DGE queue (~190GB/s): weights + xt c3;
            # sync/gpsimd hardware queues (~105GB/s each): xt c0-c2
            nc.scalar.dma_start(out=wqt[:, 0], in_=wq_d[:, 0])
            nc.scalar.dma_start(out=wkt[:, 0], in_=wk_d[:, 0])
            nc.scalar.dma_start(out=wvt[:], in_=wv_d[:])
            for c in range(3):
                for kt in range(0, 8, 2):
                    nc.sync.dma_start(out=xt[kt][c][:],
                                      in_=xt_d[:, kt, c * 512:(c + 1) * 512])
                    nc.gpsimd.dma_start(out=xt[kt + 1][c][:],
                                        in_=xt_d[:, kt + 1, c * 512:(c + 1) * 512])
            nc.scalar.dma_start(out=wqt[:, 1], in_=wq_d[:, 1])
            nc.scalar.dma_start(out=wkt[:, 1], in_=wk_d[:, 1])
            for kt in range(8):
                nc.scalar.dma_start(out=xt[kt][3][:],
                                    in_=xt_d[:, kt, 1536:2048])
            nc.scalar.dma_start(out=wot[:], in_=wo_d[:])

            # ones rows for the denominator trick (disjoint from the V
            # copies); DVE is idle during the initial DMA wait
            nc.vector.memset(vbig[:, 0:4, :, 0:64], 1.0)
            nc.vector.memset(vbig[:, 4:16, :, 0:64], 1.0)

            # pull the ~2.7us exp ACT_TABLE_LOAD off the critical path by
            # issuing a tiny dummy activation before the first real exp
            warm = keep.tile([1, 8], F32, tag="warm", name="warm")
            nc.vector.memset(warm[:], 0.0)
            nc.scalar.activation(warm[:], warm[:], EXP)

            # ------- matmul-group emit helpers (atom-granular) -------
            def qk_atoms(nm, pair, ch):
                """Project q or k for one 512-token chunk: 8 accumulating
                matmuls (one atom each) + a PSUM->SBUF cast."""
                wt = wqt if nm == "q" else wkt
                dst = qT[pair] if nm == "q" else kT[pair]
                st = {}

                def mk(kt):
                    def f():
                        if kt == 0:
                            st["ps"] = psp.tile([128, 512], F32, tag="fill",
                                                bufs=1, name="qkps")
                        nc.tensor.matmul(
                            st["ps"][:],
                            wt[:, pair, kt, :],
                            xt[kt][ch][:],
                            start=(kt == 0),
                            stop=(kt == 7),
                        )
                        if kt == 7:
                            nc.vector.tensor_copy(
                                dst[:, ch * 512:(ch + 1) * 512], st["ps"][:])
                    return f
                return [(260, mk(kt)) for kt in range(8)]

            def v_atoms(kb):
                """V projection for one 128-token key block: 2 atoms of 4
                matmuls (N=256) + cast into the [ones|v] tile."""
                st = {}

                def mk(half):
                    def f():
                        if half == 0:
                            st["ps"] = psp.tile([128, 512], F32, tag="fill",
                                                bufs=1, name="vps")
                        for kt in range(4 * half, 4 * half + 4):
                            nc.tensor.matmul(
                                st["ps"][:, 0:256],
                                xt[kt][kb // 4][:, (kb % 4) * 128:(kb % 4 + 1) * 128],
                                wvt[:, kt, :],
                                start=(kt == 0),
                                stop=(kt == 7),
                            )
                        if half == 1:
                            nc.vector.tensor_copy(
                                v_sb[kb][:, :, 64:128],
                                st["ps"][:, 0:256].rearrange(
                                    "p (a b) -> p a b", a=4))
                    return f
                return [(520, mk(0)), (520, mk(1))]

            def oproj_atoms(hg, ps_tag="fill", ps_bufs=1, out_eng=None):
                """O-projection for head group hg: 2 column halves x 8
                accumulating matmuls with stride-16 stationary reads of qs."""
                atoms = []
                qs_h = qs[:, hg, :].rearrange("p (r t) -> p r t", t=16)
                for h in range(2):
                    st = {}

                    def mk(kt, h=h, st=st):
                        def f():
                            if kt == 0:
                                st["ps"] = psp.tile([128, 512], F32, tag=ps_tag,
                                                    bufs=ps_bufs, name="ops")
                            nc.tensor.matmul(
                                st["ps"][:],
                                qs_h[:, :, 2 * kt + 1],
                                wot[:, kt, h * 512:(h + 1) * 512],
                                start=(kt == 0),
                                stop=(kt == 7),
                            )
                            if kt == 7:
                                ot = osbp.tile([128, 512], F32, tag="ot", name="ot")
                                nc.vector.tensor_copy(ot[:], st["ps"][:])
                                (out_eng or nc.sync).dma_start(
                                    out=out_d[hg, :, h * 512:(h + 1) * 512],
                                    in_=ot[:])
                        return f
                    atoms += [(260, mk(kt)) for kt in range(8)]
                return atoms

            # budget-paced filler scheduler (used from window 1 on)
            from collections import deque
            fq = deque()
            bstate = {"b": 0.0}

            def sched_add(gid, atoms):
                for c, fn in atoms:
                    fq.append((c, fn, gid))

            def sched_step(budget=600.0):
                bstate["b"] = min(bstate["b"] + budget, 1300.0)
                while fq and fq[0][0] <= bstate["b"]:
                    c, fn, _ = fq.popleft()
                    bstate["b"] -= c
                    fn()

            def sched_require(*gids):
                """Force-emit queued groups up to and including the given
                gids (compile-order backstop: the PE stream is in-order, so
                prerequisites must be emitted before their consumers)."""
                want = set(gids)
                while want & {g for _, _, g in fq}:
                    c, fn, g = fq.popleft()
                    fn()

            def sched_flush():
                while fq:
                    fq.popleft()[1]()
                bstate["b"] = 0.0

            # ------- attention window -------
            def attn_window(pair, qc, w0_fillers=None, last=False):
                q0 = qc * 512
                av = {hl: psp.tile([128, 512], F32, tag="av", bufs=3,
                                   name=f"av{hl}")
                      for hl in range(2)}
                prev_et = None

                def av_mms(kb, et):
                    for hl in range(2):
                        nc.tensor.matmul(
                            av[hl][:],
                            v_sb[kb][:, 2 * pair + hl, :],
                            et[:, hl * 512:(hl + 1) * 512],
                            start=(kb == 0),
                            stop=(kb == 15),
                        )

                for kb in range(16):
                    sc = psp.tile([128, 1024], F32, tag="sc", bufs=2, name="sc")
                    for hl in range(2):
                        h0 = hl * 64
                        nc.tensor.matmul(
                            sc[:, hl * 512:(hl + 1) * 512],
                            kT[pair][h0:h0 + 64, kb * 128:(kb + 1) * 128],
                            qT[pair][h0:h0 + 64, q0:q0 + 512],
                            start=True,
                            stop=True,
                        )
                    if prev_et is not None:
                        av_mms(kb - 1, prev_et)
                    et = expp.tile([128, 1024], BF16, tag="exp", name="et")
                    nc.scalar.activation(et[:], sc[:], EXP)
                    prev_et = et
                    if w0_fillers is not None:
                        for fn in w0_fillers.get(kb, []):
                            fn()
                    else:
                        sched_step()
                av_mms(15, prev_et)

                # window drain: normalize into qs + shifted lower copy
                n = 511 if qc == 3 else 512
                if not last:
                    for hl in range(2):
                        hg = 2 * pair + hl
                        ap = av[hl]
                        rt = rcpp.tile([128, 512], F32, tag="rcp", name="rt")
                        nc.vector.reciprocal_approx_fast(rt[0:64, :], ap[0:64, :])
                        nc.sync.dma_start(out=rt[64:128, :], in_=rt[0:64, :])
                        nc.vector.tensor_mul(
                            qs[64:128, hg, q0:q0 + 512],
                            ap[64:128, :],
                            rt[64:128, :],
                        )
                        nc.gpsimd.dma_start(
                            out=qs[0:64, hg, q0 + 1:q0 + 1 + n],
                            in_=qs[64:128, hg, q0:q0 + n],
                        )
                else:
                    # final window: interleave the two heads' drain chains
                    # (rcp -> broadcast -> mul -> shift) across engines and
                    # start each head's O-projection as soon as its own qs
                    # writes are issued
                    rts = {}
                    for hl in range(2):
                        rts[hl] = rcpp.tile([128, 512], F32, tag="rcp", name="rt")
                        nc.vector.reciprocal_approx_fast(
                            rts[hl][0:64, :], av[hl][0:64, :])
                        eng = nc.scalar if hl == 0 else nc.sync
                        eng.dma_start(out=rts[hl][64:128, :], in_=rts[hl][0:64, :])
                    for hl in range(2):
                        hg = 2 * pair + hl
                        nc.vector.tensor_mul(
                            qs[64:128, hg, q0:q0 + 512],
                            av[hl][64:128, :],
                            rts[hl][64:128, :],
                        )
                        eng = nc.gpsimd if hl == 0 else nc.sync
                        eng.dma_start(
                            out=qs[0:64, hg, q0 + 1:q0 + 1 + n],
                            in_=qs[64:128, hg, q0:q0 + n],
                        )
                        for _, fn in oproj_atoms(hg, ps_tag="sc", ps_bufs=2,
                                                 out_eng=nc.scalar):
                            fn()

            # ------- pre-phase: minimum to start pair-0 window 0 -------
            # (fill-ring groups must never interleave: each group's atoms
            # are emitted contiguously)
            for _, fn in qk_atoms("q", 0, 0):
                fn()
            for _, fn in qk_atoms("k", 0, 0):
                fn()

            # ------- window 0 of pair 0: deadline-driven fillers -------
            # v(kb) must land before iter kb+1; kT chunk c before iter 4c;
            # qT chunk 1 before window 1. Whole groups per iteration.
            w0 = {kb: [] for kb in range(16)}
            w0[0] = [v_atoms(0), v_atoms(1), v_atoms(2)]
            w0[1] = [v_atoms(3), v_atoms(4)]
            w0[2] = [v_atoms(5)]
            w0[3] = [qk_atoms("k", 0, 1)]
            w0[4] = [v_atoms(6), v_atoms(7)]
            w0[5] = [v_atoms(8), v_atoms(9)]
            w0[6] = [v_atoms(10)]
            w0[7] = [qk_atoms("k", 0, 2)]
            w0[8] = [v_atoms(11), v_atoms(12)]
            w0[9] = [v_atoms(13)]
            w0[10] = [v_atoms(14)]
            w0[11] = [qk_atoms("k", 0, 3)]
            w0[12] = [v_atoms(15)]
            w0[14] = [qk_atoms("q", 0, 1)]
            w0f = {kb: [fn for grp in lst for _, fn in grp]
                   for kb, lst in w0.items()}
            attn_window(0, 0, w0_fillers=w0f)

            # ------- windows 1..7: budget-paced fillers, deadline order ----
            sched_add("q02", qk_atoms("q", 0, 2))    # by iter 32
            sched_add("k10", qk_atoms("k", 1, 0))    # by iter 64
            sched_add("q03", qk_atoms("q", 0, 3))    # by iter 48
            sched_add("k11", qk_atoms("k", 1, 1))    # by iter 68
            sched_add("k12", qk_atoms("k", 1, 2))    # by iter 72
            sched_add("k13", qk_atoms("k", 1, 3))    # by iter 76
            sched_add("q10", qk_atoms("q", 1, 0))    # by iter 64
            sched_add("q11", qk_atoms("q", 1, 1))    # by iter 80
            sched_add("q12", qk_atoms("q", 1, 2))    # by iter 96
            sched_add("q13", qk_atoms("q", 1, 3))    # by iter 112

            attn_window(0, 1)
            sched_require("q02")
            attn_window(0, 2)
            sched_require("q03")
            attn_window(0, 3)
            # pair-0 qs complete after the drains above -> O-proj becomes
            # available as filler work during pair 1.
            sched_require("k10", "k11", "k12", "k13", "q10")
            sched_add("op0", oproj_atoms(0))
            sched_add("op1", oproj_atoms(1))
            attn_window(1, 0)
            sched_require("q11")
            attn_window(1, 1)
            sched_require("q12")
            attn_window(1, 2)
            sched_require("q13")
            sched_flush()
            attn_window(1, 3, last=True)

    nc.compile()
    return nc


def _get_program():
    if "nc" not in _CACHE:
        _CACHE["nc"] = _build_program()
    return _CACHE["nc"]


def _make_in_maps(x, Wq, Wk, Wv, Wo):
    import ml_dtypes

    bf16 = ml_dtypes.bfloat16
    wo8 = np.ascontiguousarray(
        Wo.astype(bf16).reshape(8, 128, 1024).transpose(1, 0, 2))
    xts = [
        np.ascontiguousarray(
            x[b].T.astype(np.float16).reshape(8, 128, 2048).transpose(1, 0, 2))
        for b in range(B)
    ]
    wq16 = Wq.astype(np.float16)
    wk16 = Wk.astype(np.float16)
    wv16 = Wv.astype(np.float16)
    def pack(w, cols):
        return np.ascontiguousarray(
            w[:, cols].reshape(8, 128, 256).transpose(1, 0, 2))
    def pack_qk(w, cols):
        return np.ascontiguousarray(
            w[:, cols].reshape(8, 128, 2, 128).transpose(1, 2, 0, 3))
    in_maps = []
    for c in range(NCORES):
        b, g = c // 4, c % 4
        cols = slice(4 * g * DH, 4 * (g + 1) * DH)
        in_maps.append(
            {
                "xt": xts[b],
                "wq": pack_qk(wq16, cols),
                "wk": pack_qk(wk16, cols),
                "wv": pack(wv16, cols),
                "wo": wo8,
            }
        )
    return in_maps


def kernel(x, Wq, Wk, Wv, Wo, n_head):
    from concourse.bass_utils import run_bass_kernel_spmd

    assert int(n_head) == H
    x = np.asarray(x, np.float32)
    Wq = np.asarray(Wq, np.float32)
    Wk = np.asarray(Wk, np.float32)
    Wv = np.asarray(Wv, np.float32)
    Wo = np.asarray(Wo, np.float32)

    nc = _get_program()
    in_maps = _make_in_maps(x, Wq, Wk, Wv, Wo)
    res = run_bass_kernel_spmd(nc, in_maps, list(range(NCORES)))

    out = np.empty((B, S, D), np.float32)
    for c in range(NCORES):
        b, g = c // 4, c % 4
        out[b, g * 512:(g + 1) * 512, :] = res.results[c]["out"].reshape(512, 1024)
    return out
